# revision 15
# baseline (speedup 1.0000x reference)
"""Multi-head attention (B=2, S=2048, D=1024, H=16) on 8 trn2 NeuronCores.

Sharding: head-parallel. Core c owns heads (2c, 2c+1): it computes
q/k/v projections for its 128 output dims (W_{q,k,v} column slice),
full attention for its heads over both batches, and the partial
output projection (W_o row slice). Host sums the 8 partial outputs.

Pipelined chunk structure: for each 512-token chunk, project q/k/v
for that chunk, run attention for the chunk (which, causally, only
needs k/v of chunks <= it), normalize, and emit its output
projection. Tile-level dependencies let projection DMAs/matmuls of
chunk N+1 overlap attention of chunk N, keeping PE dense (HAM warm).

Per-core layout (batch b, head h):
  q/k chunk tiles [dk=64 on partitions, t 512 free] (2 heads = 128 parts)
  vaug group [j 128, jo 4, h 2, 65]: v in natural [j, dk] + ones col 64
  scoresT [j 128, h 2, i 512] one PSUM tile (2 banks; heads run
     concurrently in PE via tile_position row tiling, K=64)
  P = exp(scores/8) bf16, both heads in ONE ACT instruction;
     block masking: skipped 128-blocks zeroed, mixed multiplied by mask
  avT[65, i] += vaug[j_t,h].T @ P_h accumulated over j_t in PSUM;
     row 64 = softmax denominator (ones column)
  normalize: DVE reciprocal of denom row + gpsimd partition_broadcast,
     then (av * recip) -> ot chunk tile; out-projection vs W_o slice.
"""

import sys

for _p in ("/opt/trn_rl_repo",):
    if _p not in sys.path:
        sys.path.insert(0, _p)

import numpy as np
import ml_dtypes

import concourse.bass as bass
import concourse.tile as tile
from concourse import bacc, mybir
from concourse.bass_utils import run_bass_kernel_spmd

B, S, D, H, DK = 2, 2048, 1024, 16, 64
T = B * S  # 4096 tokens flattened
NCORES = 8
P = 128
NT = S // P  # 16 j/i tiles per batch
NCHUNK = 4  # i-chunks of 512 per batch
CW = 512  # chunk width

bf16 = mybir.dt.bfloat16
f32 = mybir.dt.float32
nbf16 = ml_dtypes.bfloat16

_CACHE: dict = {}


def _classify_mask(m: np.ndarray):
    """m: [S, S] int (m[i, j] != 0 -> position j visible to query i).

    Returns per-i-chunk schedule in scoresT orientation (j on partitions,
    i on free):
      sched[ci] = list of (j_t, span_lo_it, memset_its, [(i_t, pat_idx)])
      first_jt[ci] = j_t whose AV matmul carries start=True
      patterns: [npat, 128, 128] bf16 mask blocks, maskblk[p][j, i]
    """
    mb = (np.asarray(m) != 0).reshape(NT, P, NT, P)  # [i_t, i, j_t, j]
    ball = mb.all(axis=(1, 3))  # [i_t, j_t]
    bany = mb.any(axis=(1, 3))

    patterns = []
    pat_idx: dict = {}
    sched = []
    first_jt = []
    for ci in range(NCHUNK):
        its = range(ci * 4, ci * 4 + 4)
        entries = []
        for j_t in range(NT):
            stat = [(bany[i_t, j_t], ball[i_t, j_t]) for i_t in its]
            if not any(a for a, _ in stat):
                continue
            nz = [k for k, (a, _) in enumerate(stat) if a]
            span_lo = nz[0]
            memsets = []
            mixeds = []
            for k in range(span_lo, 4):
                i_t = ci * 4 + k
                a, al = stat[k]
                if not a:
                    memsets.append(k)
                elif not al:
                    blk = mb[i_t, :, j_t, :].T.astype(nbf16)  # [j, i]
                    key = blk.tobytes()
                    if key not in pat_idx:
                        pat_idx[key] = len(patterns)
                        patterns.append(blk)
                    mixeds.append((k, pat_idx[key]))
            entries.append((j_t, span_lo, memsets, mixeds))
        sched.append(entries)
        first_jt.append(entries[0][0] if entries else None)

    if patterns:
        pats = np.stack(patterns)  # [npat, 128, 128]
    else:
        pats = np.ones((1, P, P), dtype=nbf16)
    return sched, first_jt, pats


def _build(sched, first_jt, npat, use_bv, use_bqk):
    nc = bacc.Bacc("TRN2", target_bir_lowering=False, debug=False,
                   num_devices=NCORES)

    qt = nc.dram_tensor("qt", [D, T], bf16, kind="ExternalInput").ap()
    kt = nc.dram_tensor("kt", [D, T], bf16, kind="ExternalInput").ap()
    vt = nc.dram_tensor("vt", [D, T], bf16, kind="ExternalInput").ap()
    wq = nc.dram_tensor("wq", [D, P], bf16, kind="ExternalInput").ap()
    wk = nc.dram_tensor("wk", [D, P], bf16, kind="ExternalInput").ap()
    wv = nc.dram_tensor("wv", [D, P], bf16, kind="ExternalInput").ap()
    wo = nc.dram_tensor("wo", [P, D], bf16, kind="ExternalInput").ap()
    bq = nc.dram_tensor("bq", [P, 1], f32, kind="ExternalInput").ap()
    bk = nc.dram_tensor("bk", [P, 1], f32, kind="ExternalInput").ap()
    bv = nc.dram_tensor("bv", [P, 1], f32, kind="ExternalInput").ap()
    maskp = nc.dram_tensor("maskp", [P, npat, P], bf16,
                           kind="ExternalInput").ap()
    out = nc.dram_tensor("out", [T, D], bf16, kind="ExternalOutput").ap()

    KC = D // P  # 8 contraction chunks for projections
    scale = 1.0 / float(np.sqrt(DK))

    with tile.TileContext(nc) as tc:
        with (
            tc.tile_pool(name="wp", bufs=1) as wp,
            tc.tile_pool(name="xt", bufs=16) as xp,
            tc.tile_pool(name="qk", bufs=NCHUNK + 1) as qkp,
            tc.tile_pool(name="kc", bufs=NCHUNK + 1) as kcp,
            tc.tile_pool(name="vg", bufs=NCHUNK + 1) as vgp,
            tc.tile_pool(name="pt", bufs=3) as ptp,
            tc.tile_pool(name="otc", bufs=2) as otp,
            tc.tile_pool(name="norm", bufs=2) as nmp,
            tc.tile_pool(name="outp", bufs=4) as obp,
            tc.tile_pool(name="pps", bufs=2, space="PSUM") as pps,
            tc.tile_pool(name="scps", bufs=2, space="PSUM") as scp,
            tc.tile_pool(name="avps", bufs=1, space="PSUM") as avp,
        ):
            # ---- persistent tiles ----
            wq_sb = wp.tile([P, KC, P], bf16, tag="wq")
            wk_sb = wp.tile([P, KC, P], bf16, tag="wk")
            wv_sb = wp.tile([P, KC, P], bf16, tag="wv")
            wo_sb = wp.tile([P, D], bf16, tag="wo")
            nc.sync.dma_start(wq_sb[:], wq.rearrange("(c p) m -> p c m", p=P))
            nc.sync.dma_start(wk_sb[:], wk.rearrange("(c p) m -> p c m", p=P))
            nc.sync.dma_start(wv_sb[:], wv.rearrange("(c p) m -> p c m", p=P))
            nc.sync.dma_start(wo_sb[:], wo[:])
            if use_bqk:
                bq_sb = wp.tile([P, 1], f32, tag="bq")
                bk_sb = wp.tile([P, 1], f32, tag="bk")
                nc.sync.dma_start(bq_sb[:], bq[:])
                nc.sync.dma_start(bk_sb[:], bk[:])
            if use_bv:
                bv_sb = wp.tile([P, 1], f32, tag="bv")
                nc.sync.dma_start(bv_sb[:], bv[:])
            mk_sb = wp.tile([P, npat, P], bf16, tag="mk")
            nc.sync.dma_start(mk_sb[:], maskp[:])

            k_tiles: dict = {}
            v_tiles: dict = {}
            q_tiles: dict = {}

            def project_chunk(tck):
                """Project q/k/v for token chunk tck into fresh tiles."""
                t0 = tck * CW
                qc = qkp.tile([P, CW], bf16, tag="qc", name=f"qc_{tck}")
                kc_t = kcp.tile([P, CW], bf16, tag="kc", name=f"kc_{tck}")
                # vaug group: [j, jo, h, 65]; col 64 = ones
                vg = vgp.tile([P, 4, 2, 65], bf16, tag="vg", name=f"vg_{tck}")
                nc.vector.memset(vg[:, :, :, 64], 1.0)
                q_tiles[tck] = qc
                k_tiles[tck] = kc_t
                v_tiles[tck] = vg

                for pi, (src, w_sb, dst) in enumerate(
                        ((qt, wq_sb, qc), (kt, wk_sb, kc_t))):
                    ps = pps.tile([P, CW], f32, tag="proj",
                                  name=f"proj{pi}_{tck}")
                    for kci in range(KC):
                        xt_t = xp.tile([P, CW], bf16, tag="xt",
                                       name=f"xt{pi}_{tck}_{kci}")
                        nc.sync.dma_start(
                            xt_t[:], src[kci * P:(kci + 1) * P, t0:t0 + CW])
                        nc.tensor.matmul(ps[:], w_sb[:, kci, :], xt_t[:],
                                         start=(kci == 0),
                                         stop=(kci == KC - 1))
                    if use_bqk:
                        nc.scalar.activation(
                            dst[:], ps[:],
                            mybir.ActivationFunctionType.Identity,
                            bias=(bq_sb if pi == 0 else bk_sb)[:])
                    else:
                        nc.scalar.copy(dst[:], ps[:])

                vt_tiles = []
                for kci in range(KC):
                    vt_t = xp.tile([P, CW], bf16, tag="xt",
                                   name=f"xtv_{tck}_{kci}")
                    nc.sync.dma_start(
                        vt_t[:], vt[kci * P:(kci + 1) * P, t0:t0 + CW])
                    vt_tiles.append(vt_t)
                for jo in range(4):
                    ps = pps.tile([P, CW], f32, tag="proj",
                                  name=f"projv_{tck}_{jo}")
                    for kci in range(KC):
                        nc.tensor.matmul(
                            ps[:, 0:P],
                            vt_tiles[kci][:, jo * P:(jo + 1) * P],
                            wv_sb[:, kci, :],
                            start=(kci == 0), stop=(kci == KC - 1))
                    nc.vector.tensor_copy(
                        vg[:, jo, :, 0:64],
                        ps[:, 0:P].rearrange("p (h d) -> p h d", h=2))

            # max k/v chunk index any attention chunk ci needs
            need = [max((e[0] // 4 for e in sched[ci]), default=ci)
                    for ci in range(NCHUNK)]

            def ensure_projected(tck_hi):
                for pc in range(min(tck_hi, B * NCHUNK - 1) + 1):
                    if pc not in k_tiles:
                        project_chunk(pc)

            for b in range(B):
                for ci in range(NCHUNK):
                    tck = b * NCHUNK + ci
                    t0 = tck * CW
                    req = b * NCHUNK + max(ci, need[ci])
                    # +1: prefetch the next chunk's projection ahead of this
                    # chunk's attention so its PSUM-freeing copybacks are not
                    # queued behind this chunk's exp flood on ACT
                    ensure_projected(req + 1)
                    qc = q_tiles[tck]

                    # ---- attention for this chunk ----
                    entries = sched[ci]
                    ot_c = otp.tile([P, CW], bf16, tag="otc",
                                    name=f"otc_{tck}")
                    if not entries:
                        nc.vector.memset(ot_c[:], 0.0)
                    else:
                        av = [avp.tile([65, CW], f32, tag=f"av{h}",
                                       name=f"av{h}_{tck}")
                              for h in range(2)]
                        last_jt = entries[-1][0]
                        for (j_t, span_lo, memsets, mixeds) in entries:
                            kct = k_tiles[b * NCHUNK + j_t // 4]
                            ksl = slice((j_t % 4) * P, (j_t % 4 + 1) * P)
                            vgt = v_tiles[b * NCHUNK + j_t // 4]
                            off = span_lo * P
                            # paired scores tile: [j, h, i] = 2 PSUM banks
                            sps = scp.tile([P, 2, CW], f32, tag="sc",
                                           name=f"sc_{tck}_{j_t}")
                            pt = ptp.tile([P, 2, CW], bf16, tag="pt",
                                          name=f"pt_{tck}_{j_t}")
                            for h in range(2):
                                hs = slice(h * 64, (h + 1) * 64)
                                nc.tensor.matmul(
                                    sps[:, h, off:CW],
                                    kct[hs, ksl], qc[hs, off:CW],
                                    start=True, stop=True,
                                    tile_position=(64 * h, 0))
                                if off > 0:
                                    nc.vector.memset(pt[:, h, 0:off], 0.0)
                            nc.scalar.activation(
                                pt[:, :, off:CW], sps[:, :, off:CW],
                                mybir.ActivationFunctionType.Exp,
                                scale=scale)
                            for h in range(2):
                                for (k, pat) in mixeds:
                                    sl = slice(k * P, (k + 1) * P)
                                    nc.vector.tensor_mul(
                                        pt[:, h, sl], pt[:, h, sl],
                                        mk_sb[:, pat, :])
                                for k in memsets:
                                    nc.vector.memset(
                                        pt[:, h, k * P:(k + 1) * P], 0.0)
                                nc.tensor.matmul(
                                    av[h][:], vgt[:, j_t % 4, h, :],
                                    pt[:, h, :],
                                    start=(j_t == first_jt[ci]),
                                    stop=(j_t == last_jt))
                        for h in range(2):
                            hs = slice(h * 64, (h + 1) * 64)
                            rc = nmp.tile([1, CW], bf16, tag="rc",
                                          name=f"rc_{tck}_{h}")
                            with nc.allow_low_precision(
                                    reason="bf16 1/denom; ~0.4% rel err ok"):
                                nc.vector.reciprocal(rc[:], av[h][64:65, :])
                            bcs = nmp.tile([64, CW], bf16, tag="bcs",
                                           name=f"bcs_{tck}_{h}")
                            nc.gpsimd.partition_broadcast(bcs[:], rc[:])
                            dst = ot_c[hs, :]
                            nc.vector.tensor_mul(dst, av[h][0:64, :], bcs[:])
                            if use_bv:
                                nc.vector.tensor_scalar_add(dst, dst,
                                                            bv_sb[hs])

                    # ---- output projection for this chunk ----
                    for il in range(4):
                        row0 = t0 + il * P
                        for nci in range(2):
                            op = avp.tile([P, CW], f32, tag=f"av{il % 2}",
                                          name=f"op_{tck}_{il}_{nci}")
                            nc.tensor.matmul(
                                op[:], ot_c[:, il * P:(il + 1) * P],
                                wo_sb[:, nci * CW:(nci + 1) * CW],
                                start=True, stop=True)
                            ob = obp.tile([P, CW], bf16, tag="ob",
                                          name=f"ob_{tck}_{il}_{nci}")
                            if (il + nci) % 2 == 0:
                                nc.scalar.copy(ob[:], op[:])
                            else:
                                nc.vector.tensor_copy(ob[:], op[:])
                            nc.sync.dma_start(
                                out[row0:row0 + P, nci * CW:(nci + 1) * CW],
                                ob[:])

    nc.compile()
    return nc


def _get_nc(mask_np, use_bv, use_bqk):
    sched, first_jt, pats = _classify_mask(mask_np)
    key_parts = [use_bv, use_bqk, pats.shape[0]]
    for ci in range(NCHUNK):
        key_parts.append(tuple(
            (j_t, lo, tuple(ms), tuple(mx)) for (j_t, lo, ms, mx) in sched[ci]))
    key = repr(key_parts)
    if key not in _CACHE:
        _CACHE[key] = _build(sched, first_jt, pats.shape[0], use_bv, use_bqk)
    # pack patterns as [128, npat, 128]
    packed = np.ascontiguousarray(pats.transpose(1, 0, 2))
    return _CACHE[key], packed


def make_in_maps(Q, K, V, W_q, b_q, W_k, b_k, W_v, b_v, W_o, b_o, mask):
    Q = np.asarray(Q, dtype=np.float32)
    K = np.asarray(K, dtype=np.float32)
    V = np.asarray(V, dtype=np.float32)
    mask_np = np.asarray(mask).reshape(S, S)
    b_q = np.asarray(b_q, dtype=np.float32)
    b_k = np.asarray(b_k, dtype=np.float32)
    b_v = np.asarray(b_v, dtype=np.float32)
    use_bv = bool(np.any(b_v != 0))
    use_bqk = bool(np.any(b_q != 0) or np.any(b_k != 0))

    nc, maskpack = _get_nc(mask_np, use_bv, use_bqk)

    qt = np.ascontiguousarray(Q.reshape(T, D).T).astype(nbf16)
    kt = np.ascontiguousarray(K.reshape(T, D).T).astype(nbf16)
    vt = np.ascontiguousarray(V.reshape(T, D).T).astype(nbf16)

    W_q = np.asarray(W_q, dtype=np.float32)
    W_k = np.asarray(W_k, dtype=np.float32)
    W_v = np.asarray(W_v, dtype=np.float32)
    W_o = np.asarray(W_o, dtype=np.float32)

    in_maps = []
    for c in range(NCORES):
        cs = slice(c * P, (c + 1) * P)
        in_maps.append({
            "qt": qt, "kt": kt, "vt": vt,
            "wq": np.ascontiguousarray(W_q[:, cs]).astype(nbf16),
            "wk": np.ascontiguousarray(W_k[:, cs]).astype(nbf16),
            "wv": np.ascontiguousarray(W_v[:, cs]).astype(nbf16),
            "wo": np.ascontiguousarray(W_o[cs, :]).astype(nbf16),
            "bq": b_q[cs].reshape(P, 1),
            "bk": b_k[cs].reshape(P, 1),
            "bv": b_v[cs].reshape(P, 1),
            "maskp": maskpack,
        })
    return nc, in_maps


def kernel(Q, K, V, W_q, b_q, W_k, b_k, W_v, b_v, W_o, b_o, mask):
    nc, in_maps = make_in_maps(Q, K, V, W_q, b_q, W_k, b_k, W_v, b_v,
                               W_o, b_o, mask)
    res = run_bass_kernel_spmd(nc, in_maps, core_ids=list(range(NCORES)))
    acc = np.zeros((T, D), dtype=np.float32)
    for c in range(NCORES):
        acc += res.results[c]["out"].astype(np.float32)
    acc += np.asarray(b_o, dtype=np.float32)[None, :]
    return acc.reshape(B, S, D)


# revision 17
# speedup vs baseline: 1.0574x; 1.0574x over previous
"""Multi-head attention (B=2, S=2048, D=1024, H=16) on 8 trn2 NeuronCores.

Sharding: head-parallel. Core c owns heads (2c, 2c+1): it computes
q/k/v projections for its 128 output dims (W_{q,k,v} column slice),
full attention for its heads over both batches, and the partial
output projection (W_o row slice). Host sums the 8 partial outputs.

Pipelined chunk structure: for each 512-token chunk, project q/k/v
for that chunk, run attention for the chunk (which, causally, only
needs k/v of chunks <= it), normalize, and emit its output
projection. Tile-level dependencies let projection DMAs/matmuls of
chunk N+1 overlap attention of chunk N, keeping PE dense (HAM warm).

Per-core layout (batch b, head h):
  q/k chunk tiles [dk=64 on partitions, t 512 free] (2 heads = 128 parts)
  vaug group [j 128, jo 4, h 2, 65]: v in natural [j, dk] + ones col 64
  scoresT [j 128, h 2, i 512] one PSUM tile (2 banks; heads run
     concurrently in PE via tile_position row tiling, K=64)
  P = exp(scores/8) bf16, both heads in ONE ACT instruction;
     block masking: skipped 128-blocks zeroed, mixed multiplied by mask
  avT[65, i] += vaug[j_t,h].T @ P_h accumulated over j_t in PSUM;
     row 64 = softmax denominator (ones column)
  normalize: DVE reciprocal of denom row + gpsimd partition_broadcast,
     then (av * recip) -> ot chunk tile; out-projection vs W_o slice.
"""

import sys

for _p in ("/opt/trn_rl_repo",):
    if _p not in sys.path:
        sys.path.insert(0, _p)

import numpy as np
import ml_dtypes

import concourse.bass as bass
import concourse.tile as tile
from concourse import bacc, mybir
from concourse.bass_utils import run_bass_kernel_spmd

B, S, D, H, DK = 2, 2048, 1024, 16, 64
T = B * S  # 4096 tokens flattened
NCORES = 8
P = 128
NT = S // P  # 16 j/i tiles per batch
NCHUNK = 4  # i-chunks of 512 per batch
CW = 512  # chunk width

bf16 = mybir.dt.bfloat16
f32 = mybir.dt.float32
nbf16 = ml_dtypes.bfloat16

_CACHE: dict = {}


def _classify_mask(m: np.ndarray):
    """m: [S, S] int (m[i, j] != 0 -> position j visible to query i).

    Returns per-i-chunk schedule in scoresT orientation (j on partitions,
    i on free):
      sched[ci] = list of (j_t, span_lo_it, memset_its, [(i_t, pat_idx)])
      first_jt[ci] = j_t whose AV matmul carries start=True
      patterns: [npat, 128, 128] bf16 mask blocks, maskblk[p][j, i]
    """
    mb = (np.asarray(m) != 0).reshape(NT, P, NT, P)  # [i_t, i, j_t, j]
    ball = mb.all(axis=(1, 3))  # [i_t, j_t]
    bany = mb.any(axis=(1, 3))

    patterns = []
    pat_idx: dict = {}
    sched = []
    first_jt = []
    for ci in range(NCHUNK):
        its = range(ci * 4, ci * 4 + 4)
        entries = []
        for j_t in range(NT):
            stat = [(bany[i_t, j_t], ball[i_t, j_t]) for i_t in its]
            if not any(a for a, _ in stat):
                continue
            nz = [k for k, (a, _) in enumerate(stat) if a]
            span_lo = nz[0]
            memsets = []
            mixeds = []
            for k in range(span_lo, 4):
                i_t = ci * 4 + k
                a, al = stat[k]
                if not a:
                    memsets.append(k)
                elif not al:
                    blk = mb[i_t, :, j_t, :].T.astype(nbf16)  # [j, i]
                    key = blk.tobytes()
                    if key not in pat_idx:
                        pat_idx[key] = len(patterns)
                        patterns.append(blk)
                    mixeds.append((k, pat_idx[key]))
            entries.append((j_t, span_lo, memsets, mixeds))
        sched.append(entries)
        first_jt.append(entries[0][0] if entries else None)

    if patterns:
        pats = np.stack(patterns)  # [npat, 128, 128]
    else:
        pats = np.ones((1, P, P), dtype=nbf16)
    return sched, first_jt, pats


def _build(sched, first_jt, npat, use_bv, use_bqk):
    nc = bacc.Bacc("TRN2", target_bir_lowering=False, debug=False,
                   num_devices=NCORES)

    qt = nc.dram_tensor("qt", [D, T], bf16, kind="ExternalInput").ap()
    kt = nc.dram_tensor("kt", [D, T], bf16, kind="ExternalInput").ap()
    vt = nc.dram_tensor("vt", [D, T], bf16, kind="ExternalInput").ap()
    wq = nc.dram_tensor("wq", [D, P], bf16, kind="ExternalInput").ap()
    wk = nc.dram_tensor("wk", [D, P], bf16, kind="ExternalInput").ap()
    wv = nc.dram_tensor("wv", [D, P], bf16, kind="ExternalInput").ap()
    wo = nc.dram_tensor("wo", [P, D], bf16, kind="ExternalInput").ap()
    bq = nc.dram_tensor("bq", [P, 1], f32, kind="ExternalInput").ap()
    bk = nc.dram_tensor("bk", [P, 1], f32, kind="ExternalInput").ap()
    bv = nc.dram_tensor("bv", [P, 1], f32, kind="ExternalInput").ap()
    maskp = nc.dram_tensor("maskp", [P, npat, P], bf16,
                           kind="ExternalInput").ap()
    out = nc.dram_tensor("out", [T, D], bf16, kind="ExternalOutput").ap()

    KC = D // P  # 8 contraction chunks for projections
    scale = 1.0 / float(np.sqrt(DK))

    with tile.TileContext(nc) as tc:
        with (
            tc.tile_pool(name="wp", bufs=1) as wp,
            tc.tile_pool(name="xt", bufs=24) as xp,
            tc.tile_pool(name="qk", bufs=NCHUNK + 1) as qkp,
            tc.tile_pool(name="kc", bufs=NCHUNK + 1) as kcp,
            tc.tile_pool(name="vg", bufs=NCHUNK + 1) as vgp,
            tc.tile_pool(name="pt", bufs=3) as ptp,
            tc.tile_pool(name="otc", bufs=2) as otp,
            tc.tile_pool(name="norm", bufs=2) as nmp,
            tc.tile_pool(name="outp", bufs=4) as obp,
            tc.tile_pool(name="pps", bufs=2, space="PSUM") as pps,
            tc.tile_pool(name="scps", bufs=2, space="PSUM") as scp,
            tc.tile_pool(name="avps", bufs=1, space="PSUM") as avp,
        ):
            # ---- persistent tiles ----
            wq_sb = wp.tile([P, KC, P], bf16, tag="wq")
            wk_sb = wp.tile([P, KC, P], bf16, tag="wk")
            wv_sb = wp.tile([P, KC, P], bf16, tag="wv")
            wo_sb = wp.tile([P, D], bf16, tag="wo")
            nc.sync.dma_start(wq_sb[:], wq.rearrange("(c p) m -> p c m", p=P))
            nc.sync.dma_start(wk_sb[:], wk.rearrange("(c p) m -> p c m", p=P))
            nc.sync.dma_start(wv_sb[:], wv.rearrange("(c p) m -> p c m", p=P))
            nc.sync.dma_start(wo_sb[:], wo[:])
            if use_bqk:
                bq_sb = wp.tile([P, 1], f32, tag="bq")
                bk_sb = wp.tile([P, 1], f32, tag="bk")
                nc.sync.dma_start(bq_sb[:], bq[:])
                nc.sync.dma_start(bk_sb[:], bk[:])
            if use_bv:
                bv_sb = wp.tile([P, 1], f32, tag="bv")
                nc.sync.dma_start(bv_sb[:], bv[:])
            mk_sb = wp.tile([P, npat, P], bf16, tag="mk")
            nc.sync.dma_start(mk_sb[:], maskp[:])

            k_tiles: dict = {}
            v_tiles: dict = {}
            q_tiles: dict = {}

            def project_chunk(tck):
                """Project q/k/v for token chunk tck into fresh tiles."""
                t0 = tck * CW
                qc = qkp.tile([P, CW], bf16, tag="qc", name=f"qc_{tck}")
                kc_t = kcp.tile([P, CW], bf16, tag="kc", name=f"kc_{tck}")
                # vaug group: [j, jo, h, 65]; col 64 = ones
                vg = vgp.tile([P, 4, 2, 65], bf16, tag="vg", name=f"vg_{tck}")
                nc.vector.memset(vg[:, :, :, 64], 1.0)
                q_tiles[tck] = qc
                k_tiles[tck] = kc_t
                v_tiles[tck] = vg

                for pi, (src, w_sb, dst) in enumerate(
                        ((qt, wq_sb, qc), (kt, wk_sb, kc_t))):
                    ps = pps.tile([P, CW], f32, tag="proj",
                                  name=f"proj{pi}_{tck}")
                    for kci in range(KC):
                        xt_t = xp.tile([P, CW], bf16, tag="xt",
                                       name=f"xt{pi}_{tck}_{kci}")
                        nc.sync.dma_start(
                            xt_t[:], src[kci * P:(kci + 1) * P, t0:t0 + CW])
                        nc.tensor.matmul(ps[:], w_sb[:, kci, :], xt_t[:],
                                         start=(kci == 0),
                                         stop=(kci == KC - 1))
                    if use_bqk:
                        nc.scalar.activation(
                            dst[:], ps[:],
                            mybir.ActivationFunctionType.Identity,
                            bias=(bq_sb if pi == 0 else bk_sb)[:])
                    else:
                        nc.scalar.copy(dst[:], ps[:])

                vt_tiles = []
                for kci in range(KC):
                    vt_t = xp.tile([P, CW], bf16, tag="xt",
                                   name=f"xtv_{tck}_{kci}")
                    nc.sync.dma_start(
                        vt_t[:], vt[kci * P:(kci + 1) * P, t0:t0 + CW])
                    vt_tiles.append(vt_t)
                for jo in range(4):
                    ps = pps.tile([P, CW], f32, tag="proj",
                                  name=f"projv_{tck}_{jo}")
                    for kci in range(KC):
                        nc.tensor.matmul(
                            ps[:, 0:P],
                            vt_tiles[kci][:, jo * P:(jo + 1) * P],
                            wv_sb[:, kci, :],
                            start=(kci == 0), stop=(kci == KC - 1))
                    nc.vector.tensor_copy(
                        vg[:, jo, :, 0:64],
                        ps[:, 0:P].rearrange("p (h d) -> p h d", h=2))

            # max k/v chunk index any attention chunk ci needs
            need = [max((e[0] // 4 for e in sched[ci]), default=ci)
                    for ci in range(NCHUNK)]

            def ensure_projected(tck_hi):
                for pc in range(min(tck_hi, B * NCHUNK - 1) + 1):
                    if pc not in k_tiles:
                        project_chunk(pc)

            for b in range(B):
                for ci in range(NCHUNK):
                    tck = b * NCHUNK + ci
                    t0 = tck * CW
                    req = b * NCHUNK + max(ci, need[ci])
                    ensure_projected(req)
                    qc = q_tiles[tck]

                    # ---- attention for this chunk ----
                    entries = sched[ci]
                    ot_c = otp.tile([P, CW], bf16, tag="otc",
                                    name=f"otc_{tck}")
                    if not entries:
                        nc.vector.memset(ot_c[:], 0.0)
                    else:
                        av = [avp.tile([65, CW], f32, tag=f"av{h}",
                                       name=f"av{h}_{tck}")
                              for h in range(2)]
                        last_jt = entries[-1][0]
                        for (j_t, span_lo, memsets, mixeds) in entries:
                            kct = k_tiles[b * NCHUNK + j_t // 4]
                            ksl = slice((j_t % 4) * P, (j_t % 4 + 1) * P)
                            vgt = v_tiles[b * NCHUNK + j_t // 4]
                            off = span_lo * P
                            # paired scores tile: [j, h, i] = 2 PSUM banks
                            sps = scp.tile([P, 2, CW], f32, tag="sc",
                                           name=f"sc_{tck}_{j_t}")
                            pt = ptp.tile([P, 2, CW], bf16, tag="pt",
                                          name=f"pt_{tck}_{j_t}")
                            for h in range(2):
                                hs = slice(h * 64, (h + 1) * 64)
                                nc.tensor.matmul(
                                    sps[:, h, off:CW],
                                    kct[hs, ksl], qc[hs, off:CW],
                                    start=True, stop=True,
                                    tile_position=(64 * h, 0))
                                if off > 0:
                                    nc.vector.memset(pt[:, h, 0:off], 0.0)
                            nc.scalar.activation(
                                pt[:, :, off:CW], sps[:, :, off:CW],
                                mybir.ActivationFunctionType.Exp,
                                scale=scale)
                            for h in range(2):
                                for (k, pat) in mixeds:
                                    sl = slice(k * P, (k + 1) * P)
                                    nc.vector.tensor_mul(
                                        pt[:, h, sl], pt[:, h, sl],
                                        mk_sb[:, pat, :])
                                for k in memsets:
                                    nc.vector.memset(
                                        pt[:, h, k * P:(k + 1) * P], 0.0)
                                nc.tensor.matmul(
                                    av[h][:], vgt[:, j_t % 4, h, :],
                                    pt[:, h, :],
                                    start=(j_t == first_jt[ci]),
                                    stop=(j_t == last_jt))
                        for h in range(2):
                            hs = slice(h * 64, (h + 1) * 64)
                            rc = nmp.tile([1, CW], bf16, tag="rc",
                                          name=f"rc_{tck}_{h}")
                            with nc.allow_low_precision(
                                    reason="bf16 1/denom; ~0.4% rel err ok"):
                                nc.vector.reciprocal(rc[:], av[h][64:65, :])
                            bcs = nmp.tile([64, CW], bf16, tag="bcs",
                                           name=f"bcs_{tck}_{h}")
                            nc.gpsimd.partition_broadcast(bcs[:], rc[:])
                            dst = ot_c[hs, :]
                            nc.vector.tensor_mul(dst, av[h][0:64, :], bcs[:])
                            if use_bv:
                                nc.vector.tensor_scalar_add(dst, dst,
                                                            bv_sb[hs])

                    # ---- output projection for this chunk ----
                    for il in range(4):
                        row0 = t0 + il * P
                        for nci in range(2):
                            op = avp.tile([P, CW], f32, tag=f"av{il % 2}",
                                          name=f"op_{tck}_{il}_{nci}")
                            nc.tensor.matmul(
                                op[:], ot_c[:, il * P:(il + 1) * P],
                                wo_sb[:, nci * CW:(nci + 1) * CW],
                                start=True, stop=True)
                            ob = obp.tile([P, CW], bf16, tag="ob",
                                          name=f"ob_{tck}_{il}_{nci}")
                            if (il + nci) % 2 == 0:
                                nc.scalar.copy(ob[:], op[:])
                            else:
                                nc.vector.tensor_copy(ob[:], op[:])
                            nc.sync.dma_start(
                                out[row0:row0 + P, nci * CW:(nci + 1) * CW],
                                ob[:])

    nc.compile()
    return nc


def _get_nc(mask_np, use_bv, use_bqk):
    sched, first_jt, pats = _classify_mask(mask_np)
    key_parts = [use_bv, use_bqk, pats.shape[0]]
    for ci in range(NCHUNK):
        key_parts.append(tuple(
            (j_t, lo, tuple(ms), tuple(mx)) for (j_t, lo, ms, mx) in sched[ci]))
    key = repr(key_parts)
    if key not in _CACHE:
        _CACHE[key] = _build(sched, first_jt, pats.shape[0], use_bv, use_bqk)
    # pack patterns as [128, npat, 128]
    packed = np.ascontiguousarray(pats.transpose(1, 0, 2))
    return _CACHE[key], packed


def make_in_maps(Q, K, V, W_q, b_q, W_k, b_k, W_v, b_v, W_o, b_o, mask):
    Q = np.asarray(Q, dtype=np.float32)
    K = np.asarray(K, dtype=np.float32)
    V = np.asarray(V, dtype=np.float32)
    mask_np = np.asarray(mask).reshape(S, S)
    b_q = np.asarray(b_q, dtype=np.float32)
    b_k = np.asarray(b_k, dtype=np.float32)
    b_v = np.asarray(b_v, dtype=np.float32)
    use_bv = bool(np.any(b_v != 0))
    use_bqk = bool(np.any(b_q != 0) or np.any(b_k != 0))

    nc, maskpack = _get_nc(mask_np, use_bv, use_bqk)

    qt = np.ascontiguousarray(Q.reshape(T, D).T).astype(nbf16)
    kt = np.ascontiguousarray(K.reshape(T, D).T).astype(nbf16)
    vt = np.ascontiguousarray(V.reshape(T, D).T).astype(nbf16)

    W_q = np.asarray(W_q, dtype=np.float32)
    W_k = np.asarray(W_k, dtype=np.float32)
    W_v = np.asarray(W_v, dtype=np.float32)
    W_o = np.asarray(W_o, dtype=np.float32)

    in_maps = []
    for c in range(NCORES):
        cs = slice(c * P, (c + 1) * P)
        in_maps.append({
            "qt": qt, "kt": kt, "vt": vt,
            "wq": np.ascontiguousarray(W_q[:, cs]).astype(nbf16),
            "wk": np.ascontiguousarray(W_k[:, cs]).astype(nbf16),
            "wv": np.ascontiguousarray(W_v[:, cs]).astype(nbf16),
            "wo": np.ascontiguousarray(W_o[cs, :]).astype(nbf16),
            "bq": b_q[cs].reshape(P, 1),
            "bk": b_k[cs].reshape(P, 1),
            "bv": b_v[cs].reshape(P, 1),
            "maskp": maskpack,
        })
    return nc, in_maps


def kernel(Q, K, V, W_q, b_q, W_k, b_k, W_v, b_v, W_o, b_o, mask):
    nc, in_maps = make_in_maps(Q, K, V, W_q, b_q, W_k, b_k, W_v, b_v,
                               W_o, b_o, mask)
    res = run_bass_kernel_spmd(nc, in_maps, core_ids=list(range(NCORES)))
    acc = np.zeros((T, D), dtype=np.float32)
    for c in range(NCORES):
        acc += res.results[c]["out"].astype(np.float32)
    acc += np.asarray(b_o, dtype=np.float32)[None, :]
    return acc.reshape(B, S, D)


# revision 19
# speedup vs baseline: 1.2884x; 1.2185x over previous
"""Multi-head attention (B=2, S=2048, D=1024, H=16) on 8 trn2 NeuronCores.

Sharding: head-parallel. Core c owns heads (2c, 2c+1): it computes
q/k/v projections for its 128 output dims (W_{q,k,v} column slice),
full attention for its heads over both batches, and the partial
output projection (W_o row slice). Host sums the 8 partial outputs.

Pipelined chunk structure: for each 512-token chunk, project q/k/v
for that chunk, run attention for the chunk (which, causally, only
needs k/v of chunks <= it), normalize, and emit its output
projection. Tile-level dependencies let projection DMAs/matmuls of
chunk N+1 overlap attention of chunk N, keeping PE dense (HAM warm).

Per-core layout (batch b, head h):
  q/k chunk tiles [dk=64 on partitions, t 512 free] (2 heads = 128 parts)
  vaug group [j 128, jo 4, h 2, 65]: v in natural [j, dk] + ones col 64
  scoresT [j 128, h 2, i 512] one PSUM tile (2 banks; heads run
     concurrently in PE via tile_position row tiling, K=64)
  P = exp(scores/8) bf16, both heads in ONE ACT instruction;
     block masking: skipped 128-blocks zeroed, mixed multiplied by mask
  avT[65, i] += vaug[j_t,h].T @ P_h accumulated over j_t in PSUM;
     row 64 = softmax denominator (ones column)
  normalize: DVE reciprocal of denom row + gpsimd partition_broadcast,
     then (av * recip) -> ot chunk tile; out-projection vs W_o slice.
"""

import sys

for _p in ("/opt/trn_rl_repo",):
    if _p not in sys.path:
        sys.path.insert(0, _p)

import numpy as np
import ml_dtypes

import concourse.bass as bass
import concourse.tile as tile
from concourse import bacc, mybir
from concourse.bass_utils import run_bass_kernel_spmd

B, S, D, H, DK = 2, 2048, 1024, 16, 64
T = B * S  # 4096 tokens flattened
NCORES = 8
P = 128
NT = S // P  # 16 j/i tiles per batch
NCHUNK = 4  # i-chunks of 512 per batch
CW = 512  # chunk width

bf16 = mybir.dt.bfloat16
f32 = mybir.dt.float32
nbf16 = ml_dtypes.bfloat16

_CACHE: dict = {}


def _classify_mask(m: np.ndarray):
    """m: [S, S] int (m[i, j] != 0 -> position j visible to query i).

    Returns per-i-chunk schedule in scoresT orientation (j on partitions,
    i on free):
      sched[ci] = list of (j_t, span_lo_it, memset_its, [(i_t, pat_idx)])
      first_jt[ci] = j_t whose AV matmul carries start=True
      patterns: [npat, 128, 128] bf16 mask blocks, maskblk[p][j, i]
    """
    mb = (np.asarray(m) != 0).reshape(NT, P, NT, P)  # [i_t, i, j_t, j]
    ball = mb.all(axis=(1, 3))  # [i_t, j_t]
    bany = mb.any(axis=(1, 3))

    patterns = []
    pat_idx: dict = {}
    sched = []
    first_jt = []
    for ci in range(NCHUNK):
        its = range(ci * 4, ci * 4 + 4)
        entries = []
        for j_t in range(NT):
            stat = [(bany[i_t, j_t], ball[i_t, j_t]) for i_t in its]
            if not any(a for a, _ in stat):
                continue
            nz = [k for k, (a, _) in enumerate(stat) if a]
            span_lo = nz[0]
            memsets = []
            mixeds = []
            for k in range(span_lo, 4):
                i_t = ci * 4 + k
                a, al = stat[k]
                if not a:
                    memsets.append(k)
                elif not al:
                    blk = mb[i_t, :, j_t, :].T.astype(nbf16)  # [j, i]
                    key = blk.tobytes()
                    if key not in pat_idx:
                        pat_idx[key] = len(patterns)
                        patterns.append(blk)
                    mixeds.append((k, pat_idx[key]))
            entries.append((j_t, span_lo, memsets, mixeds))
        sched.append(entries)
        first_jt.append(entries[0][0] if entries else None)

    if patterns:
        pats = np.stack(patterns)  # [npat, 128, 128]
    else:
        pats = np.ones((1, P, P), dtype=nbf16)
    return sched, first_jt, pats


def _build(sched, first_jt, npat, use_bv, use_bqk):
    nc = bacc.Bacc("TRN2", target_bir_lowering=False, debug=False,
                   num_devices=NCORES)

    qt = nc.dram_tensor("qt", [D, T], bf16, kind="ExternalInput").ap()
    kt = nc.dram_tensor("kt", [D, T], bf16, kind="ExternalInput").ap()
    vt = nc.dram_tensor("vt", [D, T], bf16, kind="ExternalInput").ap()
    wq = nc.dram_tensor("wq", [D, P], bf16, kind="ExternalInput").ap()
    wk = nc.dram_tensor("wk", [D, P], bf16, kind="ExternalInput").ap()
    wv = nc.dram_tensor("wv", [D, P], bf16, kind="ExternalInput").ap()
    wo = nc.dram_tensor("wo", [P, D], bf16, kind="ExternalInput").ap()
    bq = nc.dram_tensor("bq", [P, 1], f32, kind="ExternalInput").ap()
    bk = nc.dram_tensor("bk", [P, 1], f32, kind="ExternalInput").ap()
    bv = nc.dram_tensor("bv", [P, 1], f32, kind="ExternalInput").ap()
    maskp = nc.dram_tensor("maskp", [P, npat, P], bf16,
                           kind="ExternalInput").ap()
    out = nc.dram_tensor("out", [T, D], bf16, kind="ExternalOutput").ap()

    KC = D // P  # 8 contraction chunks for projections
    scale = 1.0 / float(np.sqrt(DK))

    with tile.TileContext(nc) as tc:
        with (
            tc.tile_pool(name="wp", bufs=1) as wp,
            tc.tile_pool(name="xt", bufs=24) as xp,
            tc.tile_pool(name="qk", bufs=NCHUNK + 1) as qkp,
            tc.tile_pool(name="kc", bufs=NCHUNK + 1) as kcp,
            tc.tile_pool(name="vg", bufs=NCHUNK + 1) as vgp,
            tc.tile_pool(name="pt", bufs=3) as ptp,
            tc.tile_pool(name="otc", bufs=2) as otp,
            tc.tile_pool(name="norm", bufs=2) as nmp,
            tc.tile_pool(name="outp", bufs=4) as obp,
            tc.tile_pool(name="pps", bufs=1, space="PSUM") as pps,
            tc.tile_pool(name="scps", bufs=2, space="PSUM") as scp,
            tc.tile_pool(name="avps", bufs=1, space="PSUM") as avp,
            tc.tile_pool(name="opps", bufs=1, space="PSUM") as opp,
        ):
            # ---- persistent tiles ----
            wq_sb = wp.tile([P, KC, P], bf16, tag="wq")
            wk_sb = wp.tile([P, KC, P], bf16, tag="wk")
            wv_sb = wp.tile([P, KC, P], bf16, tag="wv")
            wo_sb = wp.tile([P, D], bf16, tag="wo")
            nc.sync.dma_start(wq_sb[:], wq.rearrange("(c p) m -> p c m", p=P))
            nc.sync.dma_start(wk_sb[:], wk.rearrange("(c p) m -> p c m", p=P))
            nc.sync.dma_start(wv_sb[:], wv.rearrange("(c p) m -> p c m", p=P))
            nc.sync.dma_start(wo_sb[:], wo[:])
            if use_bqk:
                bq_sb = wp.tile([P, 1], f32, tag="bq")
                bk_sb = wp.tile([P, 1], f32, tag="bk")
                nc.sync.dma_start(bq_sb[:], bq[:])
                nc.sync.dma_start(bk_sb[:], bk[:])
            if use_bv:
                bv_sb = wp.tile([P, 1], f32, tag="bv")
                nc.sync.dma_start(bv_sb[:], bv[:])
            mk_sb = wp.tile([P, npat, P], bf16, tag="mk")
            nc.sync.dma_start(mk_sb[:], maskp[:])

            k_tiles: dict = {}
            v_tiles: dict = {}
            q_tiles: dict = {}

            def proj_stream(tck):
                """Generator projecting q/k/v for chunk tck; yields between
                emitted ops so the caller can interleave (software pipeline)."""
                t0 = tck * CW
                qc = qkp.tile([P, CW], bf16, tag="qc", name=f"qc_{tck}")
                kc_t = kcp.tile([P, CW], bf16, tag="kc", name=f"kc_{tck}")
                # vaug group: [j, jo, h, 65]; col 64 = ones
                vg = vgp.tile([P, 4, 2, 65], bf16, tag="vg", name=f"vg_{tck}")
                nc.vector.memset(vg[:, :, :, 64], 1.0)
                q_tiles[tck] = qc
                k_tiles[tck] = kc_t
                v_tiles[tck] = vg

                # issue all input DMAs for this chunk up front (deep prefetch)
                xts = {}
                for pi, src in enumerate((qt, kt, vt)):
                    for kci in range(KC):
                        xt_t = xp.tile([P, CW], bf16, tag="xt",
                                       name=f"xt{pi}_{tck}_{kci}")
                        nc.sync.dma_start(
                            xt_t[:], src[kci * P:(kci + 1) * P, t0:t0 + CW])
                        xts[pi, kci] = xt_t
                yield

                for pi, (w_sb, dst) in enumerate(
                        ((wq_sb, qc), (wk_sb, kc_t))):
                    ps = pps.tile([P, CW], f32, tag="proj",
                                  name=f"proj{pi}_{tck}")
                    for kci in range(KC):
                        nc.tensor.matmul(ps[:], w_sb[:, kci, :],
                                         xts[pi, kci][:],
                                         start=(kci == 0),
                                         stop=(kci == KC - 1))
                        yield
                    if use_bqk:
                        nc.scalar.activation(
                            dst[:], ps[:],
                            mybir.ActivationFunctionType.Identity,
                            bias=(bq_sb if pi == 0 else bk_sb)[:])
                    else:
                        nc.scalar.copy(dst[:], ps[:])
                    yield

                for jo in range(4):
                    ps = pps.tile([P, CW], f32, tag="proj",
                                  name=f"projv_{tck}_{jo}")
                    for kci in range(KC):
                        nc.tensor.matmul(
                            ps[:, 0:P],
                            xts[2, kci][:, jo * P:(jo + 1) * P],
                            wv_sb[:, kci, :],
                            start=(kci == 0), stop=(kci == KC - 1))
                        yield
                    nc.vector.tensor_copy(
                        vg[:, jo, :, 0:64],
                        ps[:, 0:P].rearrange("p (h d) -> p h d", h=2))
                    yield

            def outproj_stream(tck, ot_c):
                """Generator emitting the output projection of chunk tck."""
                t0 = tck * CW
                for il in range(4):
                    row0 = t0 + il * P
                    for nci in range(2):
                        op = opp.tile([P, CW], f32, tag="op",
                                      name=f"op_{tck}_{il}_{nci}")
                        nc.tensor.matmul(
                            op[:], ot_c[:, il * P:(il + 1) * P],
                            wo_sb[:, nci * CW:(nci + 1) * CW],
                            start=True, stop=True)
                        yield
                        ob = obp.tile([P, CW], bf16, tag="ob",
                                      name=f"ob_{tck}_{il}_{nci}")
                        if (il + nci) % 2 == 0:
                            nc.scalar.copy(ob[:], op[:])
                        else:
                            nc.vector.tensor_copy(ob[:], op[:])
                        nc.sync.dma_start(
                            out[row0:row0 + P, nci * CW:(nci + 1) * CW],
                            ob[:])
                        yield

            # max k/v chunk index any attention chunk ci needs
            need = [max((e[0] // 4 for e in sched[ci]), default=ci)
                    for ci in range(NCHUNK)]

            fillers: list = []  # [(kind, tck, gen)]
            next_proj = [0]

            def drive_fillers(steps):
                while steps > 0 and fillers:
                    kind, ptck, g = fillers[0]
                    try:
                        next(g)
                        steps -= 1
                    except StopIteration:
                        fillers.pop(0)

            def drain_proj_upto(tck_hi):
                tck_hi = min(tck_hi, B * NCHUNK - 1)
                while fillers and fillers[0][0] == "proj" \
                        and fillers[0][1] <= tck_hi:
                    for _ in fillers[0][2]:
                        pass
                    fillers.pop(0)
                while next_proj[0] <= tck_hi:
                    for _ in proj_stream(next_proj[0]):
                        pass
                    next_proj[0] += 1

            def queue_proj():
                if next_proj[0] < B * NCHUNK:
                    fillers.append(("proj", next_proj[0],
                                    proj_stream(next_proj[0])))
                    next_proj[0] += 1

            drain_proj_upto(0)
            queue_proj()

            for b in range(B):
                for ci in range(NCHUNK):
                    tck = b * NCHUNK + ci
                    req = b * NCHUNK + max(ci, need[ci])
                    drain_proj_upto(req)
                    if not fillers:
                        queue_proj()
                    qc = q_tiles[tck]

                    # ---- attention for this chunk ----
                    entries = sched[ci]
                    ot_c = otp.tile([P, CW], bf16, tag="otc",
                                    name=f"otc_{tck}")
                    if not entries:
                        nc.vector.memset(ot_c[:], 0.0)
                    else:
                        av = [avp.tile([65, CW], f32, tag=f"av{h}",
                                       name=f"av{h}_{tck}")
                              for h in range(2)]
                        last_jt = entries[-1][0]
                        nsteps = max(1, 40 // len(entries))
                        for (j_t, span_lo, memsets, mixeds) in entries:
                            kct = k_tiles[b * NCHUNK + j_t // 4]
                            ksl = slice((j_t % 4) * P, (j_t % 4 + 1) * P)
                            vgt = v_tiles[b * NCHUNK + j_t // 4]
                            off = span_lo * P
                            # paired scores tile: [j, h, i] = 2 PSUM banks
                            sps = scp.tile([P, 2, CW], f32, tag="sc",
                                           name=f"sc_{tck}_{j_t}")
                            pt = ptp.tile([P, 2, CW], bf16, tag="pt",
                                          name=f"pt_{tck}_{j_t}")
                            for h in range(2):
                                hs = slice(h * 64, (h + 1) * 64)
                                nc.tensor.matmul(
                                    sps[:, h, off:CW],
                                    kct[hs, ksl], qc[hs, off:CW],
                                    start=True, stop=True,
                                    tile_position=(64 * h, 0))
                                if off > 0:
                                    nc.vector.memset(pt[:, h, 0:off], 0.0)
                            nc.scalar.activation(
                                pt[:, :, off:CW], sps[:, :, off:CW],
                                mybir.ActivationFunctionType.Exp,
                                scale=scale)
                            for h in range(2):
                                for (k, pat) in mixeds:
                                    sl = slice(k * P, (k + 1) * P)
                                    nc.vector.tensor_mul(
                                        pt[:, h, sl], pt[:, h, sl],
                                        mk_sb[:, pat, :])
                                for k in memsets:
                                    nc.vector.memset(
                                        pt[:, h, k * P:(k + 1) * P], 0.0)
                                nc.tensor.matmul(
                                    av[h][:], vgt[:, j_t % 4, h, :],
                                    pt[:, h, :],
                                    start=(j_t == first_jt[ci]),
                                    stop=(j_t == last_jt))
                            drive_fillers(nsteps)
                        for h in range(2):
                            hs = slice(h * 64, (h + 1) * 64)
                            rc = nmp.tile([1, CW], bf16, tag="rc",
                                          name=f"rc_{tck}_{h}")
                            with nc.allow_low_precision(
                                    reason="bf16 1/denom; ~0.4% rel err ok"):
                                nc.vector.reciprocal(rc[:], av[h][64:65, :])
                            bcs = nmp.tile([64, CW], bf16, tag="bcs",
                                           name=f"bcs_{tck}_{h}")
                            nc.gpsimd.partition_broadcast(bcs[:], rc[:])
                            dst = ot_c[hs, :]
                            nc.vector.tensor_mul(dst, av[h][0:64, :], bcs[:])
                            if use_bv:
                                nc.vector.tensor_scalar_add(dst, dst,
                                                            bv_sb[hs])

                    # output projection + next-chunk projection become filler
                    fillers.append(("op", tck, outproj_stream(tck, ot_c)))
                    queue_proj()

            # drain remaining fillers
            while fillers:
                for _ in fillers[0][2]:
                    pass
                fillers.pop(0)

    nc.compile()
    return nc


def _get_nc(mask_np, use_bv, use_bqk):
    sched, first_jt, pats = _classify_mask(mask_np)
    key_parts = [use_bv, use_bqk, pats.shape[0]]
    for ci in range(NCHUNK):
        key_parts.append(tuple(
            (j_t, lo, tuple(ms), tuple(mx)) for (j_t, lo, ms, mx) in sched[ci]))
    key = repr(key_parts)
    if key not in _CACHE:
        _CACHE[key] = _build(sched, first_jt, pats.shape[0], use_bv, use_bqk)
    # pack patterns as [128, npat, 128]
    packed = np.ascontiguousarray(pats.transpose(1, 0, 2))
    return _CACHE[key], packed


def make_in_maps(Q, K, V, W_q, b_q, W_k, b_k, W_v, b_v, W_o, b_o, mask):
    Q = np.asarray(Q, dtype=np.float32)
    K = np.asarray(K, dtype=np.float32)
    V = np.asarray(V, dtype=np.float32)
    mask_np = np.asarray(mask).reshape(S, S)
    b_q = np.asarray(b_q, dtype=np.float32)
    b_k = np.asarray(b_k, dtype=np.float32)
    b_v = np.asarray(b_v, dtype=np.float32)
    use_bv = bool(np.any(b_v != 0))
    use_bqk = bool(np.any(b_q != 0) or np.any(b_k != 0))

    nc, maskpack = _get_nc(mask_np, use_bv, use_bqk)

    qt = np.ascontiguousarray(Q.reshape(T, D).T).astype(nbf16)
    kt = np.ascontiguousarray(K.reshape(T, D).T).astype(nbf16)
    vt = np.ascontiguousarray(V.reshape(T, D).T).astype(nbf16)

    W_q = np.asarray(W_q, dtype=np.float32)
    W_k = np.asarray(W_k, dtype=np.float32)
    W_v = np.asarray(W_v, dtype=np.float32)
    W_o = np.asarray(W_o, dtype=np.float32)

    in_maps = []
    for c in range(NCORES):
        cs = slice(c * P, (c + 1) * P)
        in_maps.append({
            "qt": qt, "kt": kt, "vt": vt,
            "wq": np.ascontiguousarray(W_q[:, cs]).astype(nbf16),
            "wk": np.ascontiguousarray(W_k[:, cs]).astype(nbf16),
            "wv": np.ascontiguousarray(W_v[:, cs]).astype(nbf16),
            "wo": np.ascontiguousarray(W_o[cs, :]).astype(nbf16),
            "bq": b_q[cs].reshape(P, 1),
            "bk": b_k[cs].reshape(P, 1),
            "bv": b_v[cs].reshape(P, 1),
            "maskp": maskpack,
        })
    return nc, in_maps


def kernel(Q, K, V, W_q, b_q, W_k, b_k, W_v, b_v, W_o, b_o, mask):
    nc, in_maps = make_in_maps(Q, K, V, W_q, b_q, W_k, b_k, W_v, b_v,
                               W_o, b_o, mask)
    res = run_bass_kernel_spmd(nc, in_maps, core_ids=list(range(NCORES)))
    acc = np.zeros((T, D), dtype=np.float32)
    for c in range(NCORES):
        acc += res.results[c]["out"].astype(np.float32)
    acc += np.asarray(b_o, dtype=np.float32)[None, :]
    return acc.reshape(B, S, D)
